# revision 25
# baseline (speedup 1.0000x reference)
"""Trainium2 bass kernel for nn_CM_41162966565199 (dense_cnn, dynamic filter).

Computation (per batch sample):
  filt = Conv2d(C=64 -> 9C=576, 3x3, pad=1)(gt) + bias          # dynamic filters
  out[c,h,w] = sum_j filt[c*9+j, h, w] * patches_j(gr)[c, h, w] # 3x3 dyn. filter

Strategy: pure data parallel, one sample per NeuronCore (N=8, 8 cores).

Per core:
- Conv as shift-based matmuls in fp16 (full PE rate, half the DMA bytes of
  fp32r; conv operand rounding adds ~1e-3 rel err vs the 2e-2 budget):
  contraction (in_channel i, tap p) tiled into 5 K=128 chunks by pairing taps
  whose flat-offset delta is +1 (or +132), realized by stacking two shifted
  copies of gt on SBUF partitions 0-63 / 64-127. Output channels (c, j) tiled
  into 5 M-tiles of two j-groups each. All matmuls K=128, M=128, N=512.
- Dynamic-filter stage on DVE: scalar_tensor_tensor fuses (psum + bias) * gr
  reading PSUM directly; products accumulated pairwise in fp16 (2x_1p mode);
  the upper/lower partition halves hold disjoint partial sums, folded on host.
- Spatial flattening uses a 2-ring padded 132x132 grid so every 3x3 tap is a
  pure flat offset. The output grid starts at the first real pixel (grid
  offset 133), so exactly 33 N-tiles of 512 cover all 128x132-strided rows.
- Graduated block sizes (1,2,4,...,4,2): a small first block gets the PE
  started ~12us earlier; a small last block plus an interleaved add tree
  shortens the post-matmul tail.
"""

import numpy as np
import ml_dtypes

import concourse.bass as bass
import concourse.mybir as mybir
import concourse.tile as tile
from concourse import bacc
from concourse.bass_utils import run_bass_kernel_spmd
from concourse.vector_clock import ScopedClock

# ---------------------------------------------------------------- constants
N, C, H, W, KS = 8, 64, 128, 128, 3
W2 = W + 4                      # 132: 2-ring padded row width
NROW = H + 4                    # 132 padded rows
BASE = W2 + 1                   # 133: grid offset of out pixel (0,0)
NTILE = 512
NT = 33                         # N-tiles: 33*512 = 16896 >= 127*132+128
OUT_LEN = NT * NTILE            # 16896
FLAT_SRC = 18944                # padded flat source length (covers max reads)
WINW = 4 * NTILE + 272          # max rhs window width per block

F32 = mybir.dt.float32
F16 = mybir.dt.float16
ADD = mybir.AluOpType.add
MULT = mybir.AluOpType.mult

# 5 K-chunks over the 9 conv taps p=(kh,kw); flat offset d_p = kh*132+kw.
# Pairs (p_a, p_b): upper/lower SBUF partition halves. Chunks 0-2 pair
# (kh,0)+(kh,1) (delta=1, gtAB buffer), chunk 3 pairs (0,2)+(1,2)
# (delta=132, gtAC buffer), chunk 4 is the lone (2,2) with zeroed lower
# weights.
CHUNKS = [((0, 0), (0, 1)), ((1, 0), (1, 1)), ((2, 0), (2, 1)),
          ((0, 2), (1, 2)), ((2, 2), None)]
# 5 M-tiles: which two j-groups (of the 9 output filter taps) share a PSUM
# tile's upper/lower 64 partitions.
MTILES = CHUNKS

# block sizes (N-tiles per block): small head for fast pipeline start,
# small tail to shorten the post-matmul drain.
SIZES = [2, 2, 4, 4, 4, 4, 4, 4, 4, 1]
assert sum(SIZES) == NT


# ------------------------------------------------- TileContext drain patch
# This walrus build rejects >2 sync-wait commands on one CTRL instruction;
# the stock TileContext tail hangs every pending sem wait on a single SP
# Drain. Split them across single-wait SP NOPs (program order on SP still
# places them before the barrier + sem reset).
def _drain_and_barrier_split(self, tick_clock, wait_clock):
    nc = self.nc
    drain_inst = nc.sync.drain()
    wait_clock.add_sem_waits(
        drain_inst.ins, ScopedClock({None: tick_clock.global_clock})
    )
    si = drain_inst.ins.sync_info
    if si is not None and len(si.on_wait) > 1:
        waits = list(si.on_wait)
        drain_inst.ins.sync_info = mybir.SyncInfo(on_wait=[waits[0]], on_update=[])
        for w in waits[1:]:
            nop = nc.sync.nop()
            nop.ins.sync_info = mybir.SyncInfo(on_wait=[w], on_update=[])
    nc.all_engine_barrier()
    assert self.sems is not None
    popped = nc._tile_sem_poison_stack.pop()
    assert popped is self._sem_poison
    # no trailing barrier: the program ends right after, and the NEFF
    # epilogue re-zeros every hw semaphore anyway; the gpsimd-side clear
    # is ordered within the Pool queue and touches a disjoint sem range
    nc.clear_and_free_semaphores(list(self.sems.allocated().values()))


tile.TileContext._drain_and_barrier = _drain_and_barrier_split


# ------------------------------------------------------------- host prep
def _prep_gt(gt):
    """[C,H,W] -> [C, FLAT_SRC] fp16 flat 132x132 grid, 2-ring zero pad."""
    buf = np.zeros((C, FLAT_SRC), np.float16)
    pad = np.zeros((C, NROW, W2), np.float32)
    pad[:, 2:2 + H, 2:2 + W] = gt
    buf[:, :NROW * W2] = pad.reshape(C, -1).astype(np.float16)
    return buf


def _prep_gr(gr):
    """[C,H,W] -> fp16 flat 132x132 grid; inner 130x130 = replicate-pad."""
    rp = np.pad(gr, ((0, 0), (1, 1), (1, 1)), mode="edge")
    pad = np.zeros((C, NROW, W2), np.float32)
    pad[:, 1:3 + H, 1:3 + W] = rp
    buf = np.zeros((C, FLAT_SRC), np.float16)
    buf[:, :NROW * W2] = pad.reshape(C, -1).astype(np.float16)
    return buf


def _jidx(j):
    return j[0] * 3 + j[1]


def _prep_w(Wc):
    """[576,64,3,3] -> fp16 [128, 25*128] lhsT blocks [(m,chunk), K, M]."""
    out = np.zeros((5, 5, 128, 128), np.float32)
    cc = np.arange(C)
    for m, (j0, j1) in enumerate(MTILES):
        for c, (pa, pb) in enumerate(CHUNKS):
            for hk, p in ((0, pa), (1, pb)):
                if p is None:
                    continue
                kh, kw = p
                for hm, j in ((0, j0), (1, j1)):
                    if j is None:
                        continue
                    blk = Wc[cc * 9 + _jidx(j), :, kh, kw]  # [c_out, i]
                    out[m, c, 64 * hk:64 * hk + 64, 64 * hm:64 * hm + 64] = blk.T
    # partition-major [128, 25*128] so the device load is one plain 2D DMA
    return np.ascontiguousarray(
        out.reshape(25, 128, 128).transpose(1, 0, 2).reshape(128, 25 * 128)
    ).astype(np.float16)


def _prep_b(bc):
    """[576] -> [128,5] per-M-tile per-partition bias (partition-major)."""
    out = np.zeros((5, 128), np.float32)
    cc = np.arange(C)
    for m, (j0, j1) in enumerate(MTILES):
        for hm, j in ((0, j0), (1, j1)):
            if j is None:
                continue
            out[m, 64 * hm:64 * hm + 64] = bc[cc * 9 + _jidx(j)]
    return np.ascontiguousarray(out.T)


# --------------------------------------------------------- bass program
def _build():
    # Bacc (not plain Bass): its finalize() -> compile() legalizes the
    # multi-wait instructions Tile emits (move_matmul_waits_to_ldweights,
    # generate_event_semaphores) which this walrus build otherwise rejects
    # with "Too many sync wait commands".
    nc = bacc.Bacc(None, target_bir_lowering=False)
    gt_src = nc.dram_tensor("gt_src", [C, FLAT_SRC], F16, kind="ExternalInput")
    gr_src = nc.dram_tensor("gr_src", [C, FLAT_SRC], F16, kind="ExternalInput")
    w_src = nc.dram_tensor("w_src", [128, 25 * 128], F16, kind="ExternalInput")
    b_src = nc.dram_tensor("b_src", [128, 5], F32, kind="ExternalInput")
    o_dst = nc.dram_tensor("o_dst", [128, OUT_LEN], F16, kind="ExternalOutput")

    blocks = []
    t0 = 0
    for nb in SIZES:
        blocks.append((t0, nb))
        t0 += nb

    with tile.TileContext(nc) as tc:
        with (
            tc.tile_pool(name="wpool", bufs=1) as wpool,
            tc.tile_pool(name="winpool", bufs=2) as winpool,
            tc.tile_pool(name="pspool", bufs=4, space="PSUM") as pspool,
            tc.tile_pool(name="filtpool", bufs=6) as filtpool,
            tc.tile_pool(name="prodpool", bufs=12) as prodpool,
            tc.tile_pool(name="accpool", bufs=6) as accpool,
        ):
            wsb = wpool.tile([128, 25 * 128], F16, name="wsb", tag="wsb")
            bias_sb = wpool.tile([128, 5], F32, name="bias_sb", tag="bias")

            # PE p-state warmup: the PE runs at 1.2GHz until ~3-4us of
            # continuous execution (measured: first ~10 matmuls take 427ns
            # instead of 213). Burn that ramp on dummy matmuls over scratch
            # SBUF while the first windows DMA in, timed to end just as the
            # real matmuls' inputs land.
            scratch = wpool.tile([128, 512], F16, name="scratch", tag="scr")
            ps_warm = pspool.tile([128, 2 * NTILE], F32, name="ps_warm",
                                  tag="ps")
            # memset on DVE: its queue starts ~1us sooner than the Pool Q7
            # path, so the PE clock ramp starts burning earlier
            nc.vector.memset(scratch[:, :], 0.0)
            # measured: dummies run 427ns (1.2GHz) apiece; ~6 of them bridge
            # the gap until the first windows land (~9.2us), the short ones
            # keep the handoff gap small without overshooting
            for _ in range(6):
                nc.tensor.matmul(
                    ps_warm[:, 0:NTILE], scratch[:, 0:128],
                    scratch[:, 0:NTILE], start=True, stop=True,
                )
            for _ in range(3):
                nc.tensor.matmul(
                    ps_warm[:, 0:128], scratch[:, 0:128],
                    scratch[:, 0:128], start=True, stop=True,
                )

            def load_weights_m(m):
                # weights + bias ride the ACT queue: at the head it is idle
                # (first psum copy comes ~6us later), so these issue in
                # parallel with SP's serial window-DGE chain
                nc.scalar.dma_start(
                    out=wsb[:, m * 640:(m + 1) * 640],
                    in_=w_src[:, m * 640:(m + 1) * 640],
                )

            def stt(out_ap, ps_ap, b_ap, gr_ap):
                nc.vector.scalar_tensor_tensor(
                    out_ap, ps_ap, b_ap, gr_ap, op0=ADD, op1=MULT
                )

            def win_load(pool, name, src, base, pair_step, wneed):
                """Partitions 0-63 <- src[base+q], 64-127 <-
                src[base+pair_step+q], as two 2D DMAs of just the columns
                this block touches."""
                t = pool.tile([128, WINW], F16, name=name, tag=name)
                nc.sync.dma_start(out=t[0:64, 0:wneed],
                                  in_=src[:, base:base + wneed])
                nc.sync.dma_start(
                    out=t[64:128, 0:wneed],
                    in_=src[:, base + pair_step:base + pair_step + wneed],
                )
                return t

            for bi, (t0, nb) in enumerate(blocks):
                T = BASE + t0 * NTILE
                wneed = nb * NTILE + 272
                gtab = win_load(winpool, "gtab", gt_src, T, 1, wneed)
                if bi == 0:
                    # head DMA order: the first matmuls (m0 c0-c2) need only
                    # gtab + w0, so those issue first; later weights and the
                    # gr windows (first needed by the m0 product stage) follow
                    load_weights_m(0)
                    load_weights_m(1)
                gtac = win_load(winpool, "gtac", gt_src, T + 2, 132, wneed)
                if bi == 0:
                    load_weights_m(4)
                    nc.scalar.dma_start(out=bias_sb[:, :], in_=b_src[:, :])
                    load_weights_m(2)
                    load_weights_m(3)
                grab = win_load(winpool, "grab", gr_src, T, 1, wneed)
                grac = win_load(winpool, "grac", gr_src, T + 2, 132, wneed)

                # Conv matmuls per M-tile (weights reused across the block's
                # N-tiles to amortize LDWEIGHTS). PSUM tiles span TWO banks
                # (two adjacent N-tiles) so the product/add stage runs
                # 1024-wide DVE ops — halves per-op overhead. Products are
                # written fp16 so the add tree hits the DVE 2x_1P mode.
                pairs = [(p, min(2, nb - p)) for p in range(0, nb, 2)]
                prods = [[None] * 5 for _ in pairs]
                accs = [None] * len(pairs)
                # M-tile order [0,1,4,2,3] + the interleaved fp16 add tree
                # (2x_1P mode): a1=p0+p1, a1[lower]+=p4 early, so only
                # stt(m3) + two adds trail the block's last matmul
                for m in (0, 1, 4, 2, 3):
                    pst = [
                        pspool.tile([128, 2 * NTILE], F32, name=f"ps{m}_{p}",
                                    tag="ps")
                        for p in range(len(pairs))
                    ]
                    for c in range(5):
                        k = m * 5 + c
                        lhsT = wsb[:, k * 128:(k + 1) * 128]
                        for tb in range(nb):
                            q = tb * NTILE
                            if c < 3:
                                rhs = gtab[:, q + c * W2: q + c * W2 + NTILE]
                            elif c == 3:
                                rhs = gtac[:, q: q + NTILE]
                            else:
                                rhs = gtab[:, q + 266: q + 266 + NTILE]
                            out_ps = pst[tb // 2][:, (tb % 2) * NTILE:
                                                  (tb % 2 + 1) * NTILE]
                            nc.tensor.matmul(
                                out_ps, lhsT, rhs,
                                start=(c == 0), stop=(c == 4),
                            )
                    for pi, (p0, pw) in enumerate(pairs):
                        q = p0 * NTILE
                        Wd = pw * NTILE
                        pr = prodpool.tile(
                            [128, 2 * NTILE], F16, name=f"m{m}", tag="prod"
                        )
                        prods[pi][m] = pr
                        lo = 64 if m == 4 else 128
                        b_ap = bias_sb[0:lo, m:m + 1]
                        ps_ap = pst[pi][0:lo, 0:Wd]
                        if m == 3:
                            gr_ap = grac[0:lo, q: q + Wd]
                        else:
                            d = 266 if m == 4 else m * W2
                            gr_ap = grab[0:lo, q + d: q + d + Wd]
                        if bi < len(blocks) - 1 or m != 3:
                            # (psum + bias) -> fp16 on the mostly-idle ACT
                            # engine, then a pure-fp16 multiply on DVE in
                            # 2x_1P mode: cuts DVE's per-block load ~40% so
                            # it stays ahead of the PE instead of trailing
                            # it at block boundaries
                            ft = filtpool.tile(
                                [128, 2 * NTILE], F16, name=f"f{m}",
                                tag="filt")
                            nc.scalar.activation(
                                ft[0:lo, 0:Wd], ps_ap,
                                mybir.ActivationFunctionType.Identity,
                                bias=b_ap, scale=1.0)
                            nc.vector.tensor_tensor(
                                pr[0:lo, 0:Wd], ft[0:lo, 0:Wd], gr_ap,
                                op=MULT)
                        else:
                            # last block's final product: keep the fused stt
                            # on DVE — the post-last-matmul chain is one stt
                            # + two adds, shorter than copy->mult through
                            # two engines
                            stt(pr[0:lo, 0:Wd], ps_ap, b_ap, gr_ap)
                        if m == 1:
                            a1 = accpool.tile([128, 2 * NTILE], F16,
                                              name="a1", tag="acc")
                            nc.vector.tensor_tensor(
                                a1[:, 0:Wd], prods[pi][0][:, 0:Wd],
                                pr[:, 0:Wd], op=ADD)
                            accs[pi] = a1
                        elif m == 4:
                            a1 = accs[pi]
                            nc.vector.tensor_tensor(
                                a1[0:64, 0:Wd], a1[0:64, 0:Wd],
                                pr[0:64, 0:Wd], op=ADD)
                        elif m == 3:
                            a1 = accs[pi]
                            a2 = accpool.tile([128, 2 * NTILE], F16,
                                              name="a2", tag="acc")
                            nc.vector.tensor_tensor(
                                a2[:, 0:Wd], prods[pi][2][:, 0:Wd],
                                pr[:, 0:Wd], op=ADD)
                            a3 = accpool.tile([128, 2 * NTILE], F16,
                                              name="a3", tag="acc")
                            nc.vector.tensor_tensor(
                                a3[:, 0:Wd], a1[:, 0:Wd], a2[:, 0:Wd],
                                op=ADD)
                            t = t0 + p0
                            # out-DMA from the idle ACT queue: on the
                            # in-order SP queue it would park behind DVE
                            # adds and delay the next block's loads
                            nc.scalar.dma_start(
                                out=o_dst[:, t * NTILE: t * NTILE + Wd],
                                in_=a3[:, 0:Wd],
                            )
    nc.finalize()
    return nc


_NC = None


def _get_nc():
    global _NC
    if _NC is None:
        _NC = _build()
    return _NC


_RUN_KW = {}  # test harness can inject trace=True etc.
_LAST_RESULT = None


def kernel(gr, gt, Wc, bc):
    global _LAST_RESULT
    gr = np.ascontiguousarray(np.asarray(gr, dtype=np.float32))
    gt = np.ascontiguousarray(np.asarray(gt, dtype=np.float32))
    Wc = np.asarray(Wc, dtype=np.float32)
    bc = np.asarray(bc, dtype=np.float32)

    wb = _prep_w(Wc)
    bb = _prep_b(bc)
    in_maps = [
        {
            "gt_src": _prep_gt(gt[n]),
            "gr_src": _prep_gr(gr[n]),
            "w_src": wb,
            "b_src": bb,
        }
        for n in range(N)
    ]
    res = run_bass_kernel_spmd(
        _get_nc(), in_maps, core_ids=list(range(N)), **_RUN_KW
    )
    _LAST_RESULT = res

    hh = np.arange(H)
    cols = (hh * W2)[:, None] + np.arange(W)[None, :]
    outs = []
    for n in range(N):
        O = res.results[n]["o_dst"].astype(np.float32)
        flat = O[:64] + O[64:]
        outs.append(flat[:, cols])
    return np.stack(outs).astype(np.float32)


# revision 27
# speedup vs baseline: 1.0160x; 1.0160x over previous
"""Trainium2 bass kernel for nn_CM_41162966565199 (dense_cnn, dynamic filter).

Computation (per batch sample):
  filt = Conv2d(C=64 -> 9C=576, 3x3, pad=1)(gt) + bias          # dynamic filters
  out[c,h,w] = sum_j filt[c*9+j, h, w] * patches_j(gr)[c, h, w] # 3x3 dyn. filter

Strategy: pure data parallel, one sample per NeuronCore (N=8, 8 cores).

Per core:
- Conv as shift-based matmuls in fp16 (full PE rate, half the DMA bytes of
  fp32r; conv operand rounding adds ~1e-3 rel err vs the 2e-2 budget):
  contraction (in_channel i, tap p) tiled into 5 K=128 chunks by pairing taps
  whose flat-offset delta is +1 (or +132), realized by stacking two shifted
  copies of gt on SBUF partitions 0-63 / 64-127. Output channels (c, j) tiled
  into 5 M-tiles of two j-groups each. All matmuls K=128, M=128, N=512.
- Dynamic-filter stage split across engines: the ACT engine converts
  (psum + bias) -> fp16 (Identity activation with per-partition bias), DVE
  multiplies by the gr windows and runs the pairwise add tree entirely in
  fp16 2x_1p mode; the upper/lower partition halves hold disjoint partial
  sums, folded on host. The very last product keeps the fused
  scalar_tensor_tensor on DVE so the post-matmul tail is one stt + two adds.
- Spatial flattening uses a 2-ring padded 132x132 grid so every 3x3 tap is a
  pure flat offset. The output grid starts at the first real pixel (grid
  offset 133), so exactly 33 N-tiles of 512 cover all 128x132-strided rows.
- Graduated block sizes (2,2,4,...,4,1) plus PE clock-ramp warmup via dummy
  matmuls over scratch SBUF (the PE runs at 1.2GHz for its first ~4us of
  continuous execution; the dummies burn that while the first windows DMA
  in); M-tile order (0,1,4,2,3) so the half-tile m4 folds into the add tree
  early.
"""

import numpy as np
import ml_dtypes

import concourse.bass as bass
import concourse.mybir as mybir
import concourse.tile as tile
from concourse import bacc
from concourse.bass_utils import run_bass_kernel_spmd
from concourse.vector_clock import ScopedClock

# ---------------------------------------------------------------- constants
N, C, H, W, KS = 8, 64, 128, 128, 3
W2 = W + 4                      # 132: 2-ring padded row width
NROW = H + 4                    # 132 padded rows
BASE = W2 + 1                   # 133: grid offset of out pixel (0,0)
NTILE = 512
NT = 33                         # N-tiles: 33*512 = 16896 >= 127*132+128
OUT_LEN = NT * NTILE            # 16896
FLAT_SRC = 18944                # padded flat source length (covers max reads)
WINW = 4 * NTILE + 272          # max rhs window width per block

F32 = mybir.dt.float32
F16 = mybir.dt.float16
ADD = mybir.AluOpType.add
MULT = mybir.AluOpType.mult

# 5 K-chunks over the 9 conv taps p=(kh,kw); flat offset d_p = kh*132+kw.
# Pairs (p_a, p_b): upper/lower SBUF partition halves. Chunks 0-2 pair
# (kh,0)+(kh,1) (delta=1, gtAB buffer), chunk 3 pairs (0,2)+(1,2)
# (delta=132, gtAC buffer), chunk 4 is the lone (2,2) with zeroed lower
# weights.
CHUNKS = [((0, 0), (0, 1)), ((1, 0), (1, 1)), ((2, 0), (2, 1)),
          ((0, 2), (1, 2)), ((2, 2), None)]
# 5 M-tiles: which two j-groups (of the 9 output filter taps) share a PSUM
# tile's upper/lower 64 partitions.
MTILES = CHUNKS

# block sizes (N-tiles per block): small head for fast pipeline start,
# small tail to shorten the post-matmul drain.
SIZES = [2, 2, 4, 4, 4, 4, 4, 4, 4, 1]
assert sum(SIZES) == NT


# ------------------------------------------------- TileContext drain patch
# This walrus build rejects >2 sync-wait commands on one CTRL instruction;
# the stock TileContext tail hangs every pending sem wait on a single SP
# Drain. Split them across single-wait SP NOPs (program order on SP still
# places them before the barrier + sem reset).
def _drain_and_barrier_split(self, tick_clock, wait_clock):
    nc = self.nc
    drain_inst = nc.sync.drain()
    wait_clock.add_sem_waits(
        drain_inst.ins, ScopedClock({None: tick_clock.global_clock})
    )
    si = drain_inst.ins.sync_info
    if si is not None and len(si.on_wait) > 1:
        waits = list(si.on_wait)
        drain_inst.ins.sync_info = mybir.SyncInfo(on_wait=[waits[0]], on_update=[])
        for w in waits[1:]:
            nop = nc.sync.nop()
            nop.ins.sync_info = mybir.SyncInfo(on_wait=[w], on_update=[])
    nc.all_engine_barrier()
    assert self.sems is not None
    popped = nc._tile_sem_poison_stack.pop()
    assert popped is self._sem_poison
    # no trailing barrier: the program ends right after, and the NEFF
    # epilogue re-zeros every hw semaphore anyway; the gpsimd-side clear
    # is ordered within the Pool queue and touches a disjoint sem range
    nc.clear_and_free_semaphores(list(self.sems.allocated().values()))


tile.TileContext._drain_and_barrier = _drain_and_barrier_split


# ------------------------------------------------------------- host prep
def _prep_gt(gt):
    """[C,H,W] -> [C, FLAT_SRC] fp16 flat 132x132 grid, 2-ring zero pad."""
    buf = np.zeros((C, FLAT_SRC), np.float16)
    pad = np.zeros((C, NROW, W2), np.float32)
    pad[:, 2:2 + H, 2:2 + W] = gt
    buf[:, :NROW * W2] = pad.reshape(C, -1).astype(np.float16)
    return buf


def _prep_gr(gr):
    """[C,H,W] -> fp16 flat 132x132 grid; inner 130x130 = replicate-pad."""
    rp = np.pad(gr, ((0, 0), (1, 1), (1, 1)), mode="edge")
    pad = np.zeros((C, NROW, W2), np.float32)
    pad[:, 1:3 + H, 1:3 + W] = rp
    buf = np.zeros((C, FLAT_SRC), np.float16)
    buf[:, :NROW * W2] = pad.reshape(C, -1).astype(np.float16)
    return buf


def _jidx(j):
    return j[0] * 3 + j[1]


def _prep_w(Wc):
    """[576,64,3,3] -> fp16 [128, 25*128] lhsT blocks [(m,chunk), K, M]."""
    out = np.zeros((5, 5, 128, 128), np.float32)
    cc = np.arange(C)
    for m, (j0, j1) in enumerate(MTILES):
        for c, (pa, pb) in enumerate(CHUNKS):
            for hk, p in ((0, pa), (1, pb)):
                if p is None:
                    continue
                kh, kw = p
                for hm, j in ((0, j0), (1, j1)):
                    if j is None:
                        continue
                    blk = Wc[cc * 9 + _jidx(j), :, kh, kw]  # [c_out, i]
                    out[m, c, 64 * hk:64 * hk + 64, 64 * hm:64 * hm + 64] = blk.T
    # partition-major [128, 25*128] so the device load is one plain 2D DMA
    return np.ascontiguousarray(
        out.reshape(25, 128, 128).transpose(1, 0, 2).reshape(128, 25 * 128)
    ).astype(np.float16)


def _prep_b(bc):
    """[576] -> [128,5] per-M-tile per-partition bias (partition-major)."""
    out = np.zeros((5, 128), np.float32)
    cc = np.arange(C)
    for m, (j0, j1) in enumerate(MTILES):
        for hm, j in ((0, j0), (1, j1)):
            if j is None:
                continue
            out[m, 64 * hm:64 * hm + 64] = bc[cc * 9 + _jidx(j)]
    return np.ascontiguousarray(out.T)


# --------------------------------------------------------- bass program
def _build():
    # Bacc (not plain Bass): its finalize() -> compile() legalizes the
    # multi-wait instructions Tile emits (move_matmul_waits_to_ldweights,
    # generate_event_semaphores) which this walrus build otherwise rejects
    # with "Too many sync wait commands".
    nc = bacc.Bacc(None, target_bir_lowering=False)
    gt_src = nc.dram_tensor("gt_src", [C, FLAT_SRC], F16, kind="ExternalInput")
    gr_src = nc.dram_tensor("gr_src", [C, FLAT_SRC], F16, kind="ExternalInput")
    w_src = nc.dram_tensor("w_src", [128, 25 * 128], F16, kind="ExternalInput")
    b_src = nc.dram_tensor("b_src", [128, 5], F32, kind="ExternalInput")
    o_dst = nc.dram_tensor("o_dst", [128, OUT_LEN], F16, kind="ExternalOutput")

    blocks = []
    t0 = 0
    for nb in SIZES:
        blocks.append((t0, nb))
        t0 += nb

    with tile.TileContext(nc) as tc:
        with (
            tc.tile_pool(name="wpool", bufs=1) as wpool,
            tc.tile_pool(name="winpool", bufs=2) as winpool,
            tc.tile_pool(name="pspool", bufs=4, space="PSUM") as pspool,
            tc.tile_pool(name="filtpool", bufs=6) as filtpool,
            tc.tile_pool(name="prodpool", bufs=12) as prodpool,
            tc.tile_pool(name="accpool", bufs=6) as accpool,
        ):
            wsb = wpool.tile([128, 25 * 128], F16, name="wsb", tag="wsb")
            bias_sb = wpool.tile([128, 5], F32, name="bias_sb", tag="bias")

            # PE p-state warmup: the PE runs at 1.2GHz until ~3-4us of
            # continuous execution (measured: first ~10 matmuls take 427ns
            # instead of 213). Burn that ramp on dummy matmuls over scratch
            # SBUF while the first windows DMA in, timed to end just as the
            # real matmuls' inputs land.
            scratch = wpool.tile([128, 512], F16, name="scratch", tag="scr")
            ps_warm = pspool.tile([128, 2 * NTILE], F32, name="ps_warm",
                                  tag="ps")
            nc.gpsimd.memset(scratch[:, :], 0.0)
            # measured: dummies run 427ns (1.2GHz) apiece; ~7 of them bridge
            # the gap until the first windows land (~10.6us), the short ones
            # keep the handoff gap small without overshooting
            for _ in range(7):
                nc.tensor.matmul(
                    ps_warm[:, 0:NTILE], scratch[:, 0:128],
                    scratch[:, 0:NTILE], start=True, stop=True,
                )
            for _ in range(3):
                nc.tensor.matmul(
                    ps_warm[:, 0:128], scratch[:, 0:128],
                    scratch[:, 0:128], start=True, stop=True,
                )

            def load_weights_m(m):
                nc.sync.dma_start(
                    out=wsb[:, m * 640:(m + 1) * 640],
                    in_=w_src[:, m * 640:(m + 1) * 640],
                )

            def stt(out_ap, ps_ap, b_ap, gr_ap):
                nc.vector.scalar_tensor_tensor(
                    out_ap, ps_ap, b_ap, gr_ap, op0=ADD, op1=MULT
                )

            def win_load(pool, name, src, base, pair_step, wneed):
                """Partitions 0-63 <- src[base+q], 64-127 <-
                src[base+pair_step+q], as two 2D DMAs of just the columns
                this block touches."""
                t = pool.tile([128, WINW], F16, name=name, tag=name)
                nc.sync.dma_start(out=t[0:64, 0:wneed],
                                  in_=src[:, base:base + wneed])
                nc.sync.dma_start(
                    out=t[64:128, 0:wneed],
                    in_=src[:, base + pair_step:base + pair_step + wneed],
                )
                return t

            for bi, (t0, nb) in enumerate(blocks):
                T = BASE + t0 * NTILE
                wneed = nb * NTILE + 272
                gtab = win_load(winpool, "gtab", gt_src, T, 1, wneed)
                if bi == 0:
                    # head DMA order: the first matmuls (m0 c0-c2) need only
                    # gtab + w0, so those issue first; later weights and the
                    # gr windows (first needed by the m0 product stage) follow
                    load_weights_m(0)
                    load_weights_m(1)
                gtac = win_load(winpool, "gtac", gt_src, T + 2, 132, wneed)
                if bi == 0:
                    load_weights_m(4)
                    nc.sync.dma_start(out=bias_sb[:, :], in_=b_src[:, :])
                    load_weights_m(2)
                    load_weights_m(3)
                grab = win_load(winpool, "grab", gr_src, T, 1, wneed)
                grac = win_load(winpool, "grac", gr_src, T + 2, 132, wneed)

                # Conv matmuls per M-tile (weights reused across the block's
                # N-tiles to amortize LDWEIGHTS). PSUM tiles span TWO banks
                # (two adjacent N-tiles) so the product/add stage runs
                # 1024-wide DVE ops — halves per-op overhead. Products are
                # written fp16 so the add tree hits the DVE 2x_1P mode.
                pairs = [(p, min(2, nb - p)) for p in range(0, nb, 2)]
                prods = [[None] * 5 for _ in pairs]
                accs = [None] * len(pairs)
                # M-tile order [0,1,4,2,3] + the interleaved fp16 add tree
                # (2x_1P mode): a1=p0+p1, a1[lower]+=p4 early, so only
                # stt(m3) + two adds trail the block's last matmul
                for m in (0, 1, 4, 2, 3):
                    pst = [
                        pspool.tile([128, 2 * NTILE], F32, name=f"ps{m}_{p}",
                                    tag="ps")
                        for p in range(len(pairs))
                    ]
                    for c in range(5):
                        k = m * 5 + c
                        lhsT = wsb[:, k * 128:(k + 1) * 128]
                        for tb in range(nb):
                            q = tb * NTILE
                            if c < 3:
                                rhs = gtab[:, q + c * W2: q + c * W2 + NTILE]
                            elif c == 3:
                                rhs = gtac[:, q: q + NTILE]
                            else:
                                rhs = gtab[:, q + 266: q + 266 + NTILE]
                            out_ps = pst[tb // 2][:, (tb % 2) * NTILE:
                                                  (tb % 2 + 1) * NTILE]
                            nc.tensor.matmul(
                                out_ps, lhsT, rhs,
                                start=(c == 0), stop=(c == 4),
                            )
                    for pi, (p0, pw) in enumerate(pairs):
                        q = p0 * NTILE
                        Wd = pw * NTILE
                        pr = prodpool.tile(
                            [128, 2 * NTILE], F16, name=f"m{m}", tag="prod"
                        )
                        prods[pi][m] = pr
                        lo = 64 if m == 4 else 128
                        b_ap = bias_sb[0:lo, m:m + 1]
                        ps_ap = pst[pi][0:lo, 0:Wd]
                        if m == 3:
                            gr_ap = grac[0:lo, q: q + Wd]
                        else:
                            d = 266 if m == 4 else m * W2
                            gr_ap = grab[0:lo, q + d: q + d + Wd]
                        if bi < len(blocks) - 1 or m != 3:
                            # (psum + bias) -> fp16 on the mostly-idle ACT
                            # engine, then a pure-fp16 multiply on DVE in
                            # 2x_1P mode: cuts DVE's per-block load ~40% so
                            # it stays ahead of the PE instead of trailing
                            # it at block boundaries
                            ft = filtpool.tile(
                                [128, 2 * NTILE], F16, name=f"f{m}",
                                tag="filt")
                            nc.scalar.activation(
                                ft[0:lo, 0:Wd], ps_ap,
                                mybir.ActivationFunctionType.Identity,
                                bias=b_ap, scale=1.0)
                            nc.vector.tensor_tensor(
                                pr[0:lo, 0:Wd], ft[0:lo, 0:Wd], gr_ap,
                                op=MULT)
                        else:
                            # last block's final product: keep the fused stt
                            # on DVE — the post-last-matmul chain is one stt
                            # + two adds, shorter than copy->mult through
                            # two engines
                            stt(pr[0:lo, 0:Wd], ps_ap, b_ap, gr_ap)
                        if m == 1:
                            a1 = accpool.tile([128, 2 * NTILE], F16,
                                              name="a1", tag="acc")
                            nc.vector.tensor_tensor(
                                a1[:, 0:Wd], prods[pi][0][:, 0:Wd],
                                pr[:, 0:Wd], op=ADD)
                            accs[pi] = a1
                        elif m == 4:
                            a1 = accs[pi]
                            nc.vector.tensor_tensor(
                                a1[0:64, 0:Wd], a1[0:64, 0:Wd],
                                pr[0:64, 0:Wd], op=ADD)
                        elif m == 3:
                            a1 = accs[pi]
                            a2 = accpool.tile([128, 2 * NTILE], F16,
                                              name="a2", tag="acc")
                            nc.vector.tensor_tensor(
                                a2[:, 0:Wd], prods[pi][2][:, 0:Wd],
                                pr[:, 0:Wd], op=ADD)
                            a3 = accpool.tile([128, 2 * NTILE], F16,
                                              name="a3", tag="acc")
                            nc.vector.tensor_tensor(
                                a3[:, 0:Wd], a1[:, 0:Wd], a2[:, 0:Wd],
                                op=ADD)
                            t = t0 + p0
                            # out-DMA from the idle ACT queue: on the
                            # in-order SP queue it would park behind DVE
                            # adds and delay the next block's loads
                            nc.scalar.dma_start(
                                out=o_dst[:, t * NTILE: t * NTILE + Wd],
                                in_=a3[:, 0:Wd],
                            )
    nc.finalize()
    return nc


_NC = None


def _get_nc():
    global _NC
    if _NC is None:
        _NC = _build()
    return _NC


_RUN_KW = {}  # test harness can inject trace=True etc.
_LAST_RESULT = None


def kernel(gr, gt, Wc, bc):
    global _LAST_RESULT
    gr = np.ascontiguousarray(np.asarray(gr, dtype=np.float32))
    gt = np.ascontiguousarray(np.asarray(gt, dtype=np.float32))
    Wc = np.asarray(Wc, dtype=np.float32)
    bc = np.asarray(bc, dtype=np.float32)

    wb = _prep_w(Wc)
    bb = _prep_b(bc)
    in_maps = [
        {
            "gt_src": _prep_gt(gt[n]),
            "gr_src": _prep_gr(gr[n]),
            "w_src": wb,
            "b_src": bb,
        }
        for n in range(N)
    ]
    res = run_bass_kernel_spmd(
        _get_nc(), in_maps, core_ids=list(range(N)), **_RUN_KW
    )
    _LAST_RESULT = res

    hh = np.arange(H)
    cols = (hh * W2)[:, None] + np.arange(W)[None, :]
    outs = []
    for n in range(N):
        O = res.results[n]["o_dst"].astype(np.float32)
        flat = O[:64] + O[64:]
        outs.append(flat[:, cols])
    return np.stack(outs).astype(np.float32)
